# revision 1
# baseline (speedup 1.0000x reference)
"""FASTLoss (PSENet/FAST text-detection loss) on 8 Trainium2 cores.

Data-parallel: 16 samples sharded 2-per-core. Each core computes per-sample
partial sums (dice inter/union terms + OHEM threshold search via on-device
bisection); host combines the tiny per-core stat vectors into the 3 scalars.

Math notes (B=batch, g=gt_text in {0,1}, m=training_mask in {0,1}):
  pos = g*m, neg = m - pos, p = sigmoid(pred_text)
  ohem = pos | (top-k negatives by p),  k = min(3*n_pos, n_neg)
  dice_text per sample: inter = sum(p*pos)
                        union = sum(p^2*pos) + T + n_pos + eps
  where T = sum of p^2 over the k highest-scoring negatives.  T is the only
  quantity needing selection; it is computed by bisecting for the k-th
  largest value of v2 = x + 100*neg (negatives live in [92,108], everything
  else in [-8,8]), then one masked-sigmoid pass; ties at the final (adjacent
  float) threshold are fixed exactly on the host via (k - C_hi)*sigmoid(t)^2.
"""

import os
import sys

import numpy as np

sys.path.insert(0, "/opt/trn_rl_repo")

import concourse.bass as bass  # noqa: E402
import concourse.tile as tile  # noqa: E402
from concourse import bacc, bass_isa, library_config, mybir  # noqa: E402
from concourse.bass_utils import run_bass_kernel_spmd  # noqa: E402

F32 = mybir.dt.float32
BF16 = mybir.dt.bfloat16
ALU = mybir.AluOpType
ACTF = mybir.ActivationFunctionType

B_PER_CORE = 2
N_CORES = 8
P = 128          # partitions
FREE = 3200      # 640*640 / 128
NITER = 10       # phase-1 subsample bisection iterations
FULLITER = 3     # phase-2 full-resolution refinement iterations

# stats tile column map (all columns are per-partition partial sums that get
# partition-all-reduced at the end; host reads row 0 of the result)
NPOS = 0      # +b   : sum(g*m)
NNEG = 2      # +b   : sum(m - g*m)
INTERT = 4    # +b   : sum(sigmoid(x)*pos)
P2POS = 6     # +b   : sum(sigmoid(x)^2*pos)
TSEL = 8      # +b   : sum(sigmoid(x)^2 * [neg & v2>=hi])
CHI = 10      # +b   : count(v2 >= hi)
LO = 12       # +b   : final bisection lo (x128, host divides)
HI = 14       # +b   : final bisection hi (x128, host divides)
IK = 16       # +b*5+c : sum(sigmoid(xk)*t*m)
UP = 26       # +b*5+c : sum(sigmoid(xk)^2*m)
UT = 36       # +b*5+c : sum(t*m)
NCOL = 64


def build_bass(stage="full", bench_iters=1, niter=NITER):
    # stage: debug ladder -- "phases" (no gpsimd custom ops), "lib" (+
    # load_library), "par" (+ final partition_all_reduce), "full".
    # bench_iters > 1 wraps the whole body in a hardware loop so device
    # time dominates the axon dispatch overhead when benchmarking.
    nc = bacc.Bacc("TRN2", target_bir_lowering=False, debug=False)

    pred = nc.dram_tensor("pred", [B_PER_CORE, 6, P, FREE], F32,
                          kind="ExternalInput").ap()
    gtt = nc.dram_tensor("gt_text", [B_PER_CORE, P, FREE], F32,
                         kind="ExternalInput").ap()
    gtk = nc.dram_tensor("gt_kernels", [B_PER_CORE, 5, P, FREE], F32,
                         kind="ExternalInput").ap()
    msk = nc.dram_tensor("training_mask", [B_PER_CORE, P, FREE], F32,
                         kind="ExternalInput").ap()
    out = nc.dram_tensor("out", [1, NCOL], F32, kind="ExternalOutput").ap()

    with tile.TileContext(nc) as tc:
        with (
            tc.tile_pool(name="pin", bufs=1) as pin,
            tc.tile_pool(name="stream", bufs=3) as stream,
            tc.tile_pool(name="work", bufs=2) as work,
            tc.tile_pool(name="psum", bufs=2, space="PSUM") as psum,
        ):
            if stage != "phases":
                nc.gpsimd.load_library(library_config.attn)
            if bench_iters > 1:
                loop_cm = tc.For_i(0, bench_iters, 1)
                loop_cm.__enter__()
            stats = pin.tile([P, NCOL], F32)
            nc.vector.memset(stats, 0.0)

            m_t = [pin.tile([P, FREE], BF16, tag=f"m{b}", name=f"m{b}")
                    for b in range(B_PER_CORE)]
            v2_t = [pin.tile([P, FREE], F32, tag=f"v2{b}", name=f"v2{b}")
                    for b in range(B_PER_CORE)]
            bis_out = pin.tile([P, FREE], BF16, tag="bis_out")

            # bisection state, all [P, 2] (col b = sample b), identical
            # values across partitions
            lo = pin.tile([P, B_PER_CORE], F32, tag="lo")
            hi = pin.tile([P, B_PER_CORE], F32, tag="hi")
            mid = pin.tile([P, B_PER_CORE], F32, tag="mid")
            ktile = pin.tile([P, B_PER_CORE], F32, tag="ktile")
            cnt = pin.tile([P, B_PER_CORE], F32, tag="cnt")
            tot = pin.tile([P, 4], F32, tag="tot")
            cmp_t = pin.tile([P, B_PER_CORE], mybir.dt.uint32, tag="cmp")

            bias100 = pin.tile([P, 1], F32, tag="bias100")
            nc.vector.memset(bias100, -100.0)

            # phase-1 subsample state: partitions 0:64 = sample0,
            # 64:128 = sample1 (striped)
            SUBF = 800
            v2s = pin.tile([P, SUBF], F32, tag="v2s")
            bis_sub = pin.tile([P, SUBF], BF16, tag="bis_sub")
            los = pin.tile([P, 1], F32, tag="los")
            his = pin.tile([P, 1], F32, tag="his")
            mids = pin.tile([P, 1], F32, tag="mids")
            ks = pin.tile([P, 1], F32, tag="ks")
            cnt_s = pin.tile([P, 1], F32, tag="cnt_s")
            cmp_s = pin.tile([P, 1], mybir.dt.uint32, tag="cmp_s")
            # matmul masks: bm = block-diagonal (own 64-group), ones128,
            # L0/L1 = broadcast-from-group masks (rows of group g = 1/64)
            bm = pin.tile([P, P], F32, tag="bm")
            ones128 = pin.tile([P, P], F32, tag="ones128")
            L0 = pin.tile([P, P], F32, tag="L0")
            L1 = pin.tile([P, P], F32, tag="L1")
            nc.vector.memset(bm, 0.0)
            nc.vector.memset(bm[0:64, 0:64], 1.0)
            nc.vector.memset(bm[64:128, 64:128], 1.0)
            nc.vector.memset(ones128, 1.0)
            nc.vector.memset(L0, 0.0)
            nc.vector.memset(L0[0:64, :], 1.0 / 64.0)
            nc.vector.memset(L1, 0.0)
            nc.vector.memset(L1[64:128, :], 1.0 / 64.0)
            nc.vector.memset(los, 92.0)
            nc.vector.memset(his, 108.0)
            nc.vector.memset(mids, 100.0)

            # masks are binary; stage the f32 DMA through a stream slot
            # and keep a bf16 copy resident (exact for 0/1 values)
            for b in range(B_PER_CORE):
                mstage = stream.tile([P, FREE], F32, tag="x", name=f"mst{b}", bufs=4)
                nc.sync.dma_start(out=mstage, in_=msk[b])
                nc.scalar.activation(out=m_t[b], in_=mstage, func=ACTF.Copy)

            # ---------------- text phase ----------------
            for b in range(B_PER_CORE):
                x = stream.tile([P, FREE], F32, tag="x", bufs=4)
                nc.sync.dma_start(out=x, in_=pred[b, 0])
                g = stream.tile([P, FREE], F32, tag="t")
                nc.sync.dma_start(out=g, in_=gtt[b])

                posm = work.tile([P, FREE], F32, tag="aux", name="posm", bufs=1)
                nc.vector.scalar_tensor_tensor(
                    out=posm, in0=g, scalar=1.0, in1=m_t[b],
                    op0=ALU.mult, op1=ALU.mult,
                    accum_out=stats[:, NPOS + b:NPOS + b + 1])
                sig = work.tile([P, FREE], F32, tag="sig")
                nc.scalar.activation(out=sig, in_=x, func=ACTF.Sigmoid)
                # inter_text partials (overwrite g; g dead after posm)
                nc.vector.scalar_tensor_tensor(
                    out=g, in0=sig, scalar=1.0, in1=posm,
                    op0=ALU.mult, op1=ALU.mult,
                    accum_out=stats[:, INTERT + b:INTERT + b + 1])
                p2 = work.tile([P, FREE], F32, tag="s2")
                nc.scalar.activation(out=p2, in_=sig, func=ACTF.Square)
                nc.vector.scalar_tensor_tensor(
                    out=sig, in0=p2, scalar=1.0, in1=posm,
                    op0=ALU.mult, op1=ALU.mult,
                    accum_out=stats[:, P2POS + b:P2POS + b + 1])
                # negm = m - posm (into posm)
                nc.vector.scalar_tensor_tensor(
                    out=posm, in0=m_t[b], scalar=1.0, in1=posm,
                    op0=ALU.mult, op1=ALU.subtract,
                    accum_out=stats[:, NNEG + b:NNEG + b + 1])
                # v2 = 100*negm + x
                nc.vector.scalar_tensor_tensor(
                    out=v2_t[b], in0=posm, scalar=100.0, in1=x,
                    op0=ALU.mult, op1=ALU.add)

            # ---- bisection chunks (emitted interleaved with K planes so
            # the serial threshold-search chain hides inside the streaming
            # phase instead of stalling the in-order DVE stream) ----
            bis_chunks = []
            if stage == "full":
                def _setup():
                    # k = min(3*n_pos, n_neg); PE fp32 matmul with ones
                    # lhsT is exact for integer-valued counts
                    tot4 = psum.tile([P, 4], F32, tag="tot4", name="tot4")
                    nc.tensor.matmul(tot4, ones128, stats[:, NPOS:NPOS + 4],
                                     start=True, stop=True)
                    nc.vector.tensor_scalar(
                        out=ktile, in0=tot4[:, 0:B_PER_CORE], scalar1=3.0,
                        scalar2=None, op0=ALU.mult)
                    nc.vector.tensor_tensor(
                        out=ktile, in0=ktile,
                        in1=tot4[:, B_PER_CORE:2 * B_PER_CORE], op=ALU.min)
                    # striped subsample targets: k/8 (1/4 stride x half
                    # the partitions)
                    nc.vector.tensor_scalar(
                        out=ks[0:64, :], in0=ktile[0:64, 0:1], scalar1=0.125,
                        scalar2=None, op0=ALU.mult)
                    nc.vector.tensor_scalar(
                        out=ks[64:128, :], in0=ktile[64:128, 1:2],
                        scalar1=0.125, scalar2=None, op0=ALU.mult)
                    nc.vector.tensor_copy(
                        v2s[0:64, :],
                        v2_t[0][0:64, :].rearrange(
                            "p (a s) -> p a s", s=4)[:, :, 0])
                    nc.vector.tensor_copy(
                        v2s[64:128, :],
                        v2_t[1][64:128, :].rearrange(
                            "p (a s) -> p a s", s=4)[:, :, 0])
                bis_chunks.append(_setup)

                def _p1_iter():
                    # count = sum((v2s >= t) && (v2s != 0)); selected values
                    # are always >= 92 so the and() equals the indicator.
                    # stt+accum is ~4.5x faster than tensor_scalar+accum.
                    nc.vector.scalar_tensor_tensor(
                        out=bis_sub, in0=v2s, scalar=mids, in1=v2s,
                        op0=ALU.is_ge, op1=ALU.logical_and,
                        accum_out=cnt_s)
                    tot_s = psum.tile([P, 1], F32, tag="tot_s",
                                      name="tot_s")
                    nc.tensor.matmul(tot_s, bm, cnt_s, start=True,
                                     stop=True)
                    nc.vector.tensor_tensor(
                        out=cmp_s, in0=tot_s, in1=ks, op=ALU.is_ge)
                    nc.vector.copy_predicated(out=los, mask=cmp_s,
                                              data=mids)
                    nc.vector.tensor_tensor(
                        out=cmp_s, in0=tot_s, in1=ks, op=ALU.is_lt)
                    nc.vector.copy_predicated(out=his, mask=cmp_s,
                                              data=mids)
                    nc.vector.tensor_tensor(out=mids, in0=los, in1=his,
                                            op=ALU.add)
                    nc.vector.tensor_scalar_mul(mids, mids, 0.5)
                bis_chunks.extend([_p1_iter] * niter)

                def _widen():
                    # un-stripe phase-1 mids into [P, 2] and widen by DELTA
                    # to cover subsample noise (~9 sigma of rank estimate)
                    DELTA = 0.072
                    mid2 = psum.tile([P, B_PER_CORE], F32, tag="mid2",
                                     name="mid2")
                    nc.tensor.matmul(mid2[:, 0:1], L0, mids, start=True,
                                     stop=True)
                    nc.tensor.matmul(mid2[:, 1:2], L1, mids, start=True,
                                     stop=True)
                    nc.vector.tensor_scalar(
                        out=lo, in0=mid2, scalar1=DELTA, scalar2=None,
                        op0=ALU.subtract)
                    nc.vector.tensor_scalar(
                        out=hi, in0=mid2, scalar1=DELTA, scalar2=None,
                        op0=ALU.add)
                    nc.vector.tensor_tensor(out=mid, in0=lo, in1=hi,
                                            op=ALU.add)
                    nc.vector.tensor_scalar_mul(mid, mid, 0.5)
                bis_chunks.append(_widen)

                def _p2_iter():
                    for b in range(B_PER_CORE):
                        nc.vector.scalar_tensor_tensor(
                            out=bis_out, in0=v2_t[b],
                            scalar=mid[:, b:b + 1], in1=v2_t[b],
                            op0=ALU.is_ge, op1=ALU.logical_and,
                            accum_out=cnt[:, b:b + 1])
                    tot_f = psum.tile([P, B_PER_CORE], F32, tag="tot_f",
                                      name="tot_f")
                    nc.tensor.matmul(tot_f, ones128, cnt, start=True,
                                     stop=True)
                    nc.vector.tensor_tensor(
                        out=cmp_t, in0=tot_f, in1=ktile, op=ALU.is_ge)
                    nc.vector.copy_predicated(out=lo, mask=cmp_t, data=mid)
                    nc.vector.tensor_tensor(
                        out=cmp_t, in0=tot_f, in1=ktile, op=ALU.is_lt)
                    nc.vector.copy_predicated(out=hi, mask=cmp_t, data=mid)
                    nc.vector.tensor_tensor(out=mid, in0=lo, in1=hi,
                                            op=ALU.add)
                    nc.vector.tensor_scalar_mul(mid, mid, 0.5)
                bis_chunks.extend([_p2_iter] * FULLITER)

                def _final(b):
                    # C_hi and T = sum sigmoid(v2-100)^2 over v2 >= hi
                    nc.vector.scalar_tensor_tensor(
                        out=bis_out, in0=v2_t[b], scalar=hi[:, b:b + 1],
                        in1=v2_t[b], op0=ALU.is_ge, op1=ALU.logical_and,
                        accum_out=stats[:, CHI + b:CHI + b + 1])
                    w = work.tile([P, FREE], F32, tag="sig", name="w")
                    nc.vector.scalar_tensor_tensor(
                        out=w, in0=v2_t[b], scalar=hi[:, b:b + 1],
                        in1=v2_t[b], op0=ALU.is_ge, op1=ALU.mult)
                    pw = work.tile([P, FREE], F32, tag="s2", name="pw")
                    nc.scalar.activation(out=pw, in_=w, func=ACTF.Sigmoid,
                                         bias=bias100)
                    nc.scalar.activation(
                        out=w, in_=pw, func=ACTF.Square,
                        accum_out=stats[:, TSEL + b:TSEL + b + 1])
                    nc.vector.tensor_copy(stats[:, LO + b:LO + b + 1],
                                          lo[:, b:b + 1])
                    nc.vector.tensor_copy(stats[:, HI + b:HI + b + 1],
                                          hi[:, b:b + 1])
                bis_chunks.append(lambda: _final(0))
                bis_chunks.append(lambda: _final(1))

            # ---------------- kernels phase (bisection interleaved) -------
            planes = [(b, c) for b in range(B_PER_CORE) for c in range(5)]
            emitted = 0
            for j, (b, c) in enumerate(planes):
                xk = stream.tile([P, FREE], F32, tag="x", name="xk", bufs=4)
                nc.sync.dma_start(out=xk, in_=pred[b, c + 1])
                t = stream.tile([P, FREE], F32, tag="t", name="tk")
                nc.sync.dma_start(out=t, in_=gtk[b, c])

                sig = work.tile([P, FREE], F32, tag="sig", name="sig")
                nc.scalar.activation(out=sig, in_=xk, func=ACTF.Sigmoid)
                s2 = work.tile([P, FREE], F32, tag="s2", name="s2")
                nc.scalar.activation(out=s2, in_=sig, func=ACTF.Square)
                j2 = b * 5 + c
                tm = work.tile([P, FREE], F32, tag="aux", name="tm", bufs=1)
                nc.vector.scalar_tensor_tensor(
                    out=tm, in0=t, scalar=1.0, in1=m_t[b],
                    op0=ALU.mult, op1=ALU.mult,
                    accum_out=stats[:, UT + j2:UT + j2 + 1])
                nc.vector.scalar_tensor_tensor(
                    out=t, in0=sig, scalar=1.0, in1=tm,
                    op0=ALU.mult, op1=ALU.mult,
                    accum_out=stats[:, IK + j2:IK + j2 + 1])
                nc.vector.scalar_tensor_tensor(
                    out=s2, in0=s2, scalar=1.0, in1=m_t[b],
                    op0=ALU.mult, op1=ALU.mult,
                    accum_out=stats[:, UP + j2:UP + j2 + 1])
                # interleave bisection chunks between planes
                target = (j + 1) * len(bis_chunks) // len(planes)
                while emitted < target:
                    bis_chunks[emitted]()
                    emitted += 1
            while emitted < len(bis_chunks):
                bis_chunks[emitted]()
                emitted += 1

            # ---------------- final reduce + output ----------------
            if stage in ("par", "full"):
                totals = pin.tile([P, NCOL], F32, tag="totals")
                nc.gpsimd.partition_all_reduce(
                    out_ap=totals, in_ap=stats, channels=P,
                    reduce_op=bass_isa.ReduceOp.add)
                nc.sync.dma_start(out=out, in_=totals[0:1, :])
            else:
                nc.sync.dma_start(out=out, in_=stats[0:1, :])
            if bench_iters > 1:
                loop_cm.__exit__(None, None, None)

    nc.compile()
    return nc


_NC_CACHE = None


def _get_nc():
    global _NC_CACHE
    if _NC_CACHE is None:
        _NC_CACHE = build_bass()
    return _NC_CACHE


def make_in_maps(pred, gt_text, gt_kernels, training_mask):
    in_maps = []
    for core in range(N_CORES):
        s = slice(core * B_PER_CORE, (core + 1) * B_PER_CORE)
        in_maps.append({
            "pred": np.ascontiguousarray(pred[s]).reshape(
                B_PER_CORE, 6, P, FREE),
            "gt_text": np.ascontiguousarray(gt_text[s]).reshape(
                B_PER_CORE, P, FREE),
            "gt_kernels": np.ascontiguousarray(gt_kernels[s]).reshape(
                B_PER_CORE, 5, P, FREE),
            "training_mask": np.ascontiguousarray(training_mask[s]).reshape(
                B_PER_CORE, P, FREE),
        })
    return in_maps


def combine(core_outs):
    """core_outs: list of 8 arrays [1, NCOL] -> (loss, loss_text, loss_k)."""
    EPS = 1e-6
    text_losses = []
    kernel_losses = []
    for o in core_outs:
        o = np.asarray(o, dtype=np.float64).reshape(NCOL)
        for b in range(B_PER_CORE):
            n_pos = o[NPOS + b]
            n_neg = o[NNEG + b]
            k = min(3.0 * n_pos, n_neg)
            c_hi = o[CHI + b]
            lo_v = o[LO + b] / P
            hi_v = o[HI + b] / P
            t_mid = 0.5 * (lo_v + hi_v) - 100.0
            s = 1.0 / (1.0 + np.exp(-t_mid))
            T = o[TSEL + b] + (k - c_hi) * s * s
            union = o[P2POS + b] + T + n_pos + EPS
            text_losses.append(1.0 - 2.0 * o[INTERT + b] / union)
            for c in range(5):
                j = b * 5 + c
                union_k = o[UP + j] + o[UT + j] + EPS
                kernel_losses.append(1.0 - 2.0 * o[IK + j] / union_k)
    loss_text = float(np.mean(text_losses))
    loss_kernels = float(np.mean(kernel_losses))
    loss = loss_kernels + 0.5 * loss_text
    return (np.float32(loss), np.float32(loss_text), np.float32(loss_kernels))


def kernel(pred, gt_text, gt_kernels, training_mask):
    nc = _get_nc()
    in_maps = make_in_maps(pred, gt_text, gt_kernels, training_mask)
    res = run_bass_kernel_spmd(nc, in_maps, core_ids=list(range(N_CORES)))
    core_outs = [res.results[i]["out"] for i in range(N_CORES)]
    return combine(core_outs)


if __name__ == "__main__":
    rng = np.random.default_rng(0)
    B, C, H, W = 16, 6, 640, 640
    pred = rng.standard_normal((B, C, H, W), dtype=np.float32)
    gt_text = (rng.random((B, 1, H, W)) > 0.9).astype(np.float32)
    gt_kernels = (rng.random((B, C - 1, H, W)) > 0.9).astype(np.float32)
    training_mask = (rng.random((B, 1, H, W)) > 0.05).astype(np.float32)
    print(kernel(pred, gt_text, gt_kernels, training_mask))



# revision 3
# speedup vs baseline: 1.5561x; 1.5561x over previous
"""FASTLoss (PSENet/FAST text-detection loss) on 8 Trainium2 cores, v3.

Data-parallel: 16 samples, 2 per core. Host stages all inputs as bf16
(binary gt/mask tensors are exact in bf16; pred rounding is ~0.4% rel,
far inside the 2e-2 tolerance) which halves HBM traffic. On-device work
is decomposed across ALL engines to keep each under the DMA roofline:

  DVE : elementwise products via tensor_tensor bf16 (2x mode) and
        threshold selects via tensor_scalar (4x mode). No DVE accums --
        the TensorScalarPtrReduce path runs 1x on HW (~3.6us/plane).
  ACT : sigmoids + 8 of the masked squares with accum_out ([P,1]
        partials, host sums over partitions).
  Pool: all 10 t*m products (gpsimd tensor_tensor, ~6.3us/plane),
        emitted ahead of the plane loop so the in-order DVE/ACT queues
        never wait on the slow engine.
  PE  : every grand-total reduction as ones-weights matmuls accumulated
        into PSUM half-rows (bases 0/32/64, halves = cols 0:256/256:512);
        host sums the DMA'd PSUM rows.
  OHEM: bisection in p-space on pn = sigmoid(x)*neg (bf16), phase-1 on a
        1/8 subsample only; final full-res pass at hi = mid + DELTA and
        the host fixes the in-gap elements via (k - C_hi) * s^2 with
        s = mid + DELTA/2 (second-order-accurate, ~1e-4 rel).

Math notes (g = gt_text, m = training_mask, both binary; p = sigmoid):
  pos = g*m, neg = m - pos
  dice_text: inter = sum(p*pos)
             union = sum(p^2*pos) + T + n_pos + eps
  T = sum(p^2 over top-k negatives by p), k = min(3*n_pos, n_neg)
  kernels (per plane c): UT = sum(t*m), IK = sum(p*t*m), UP = sum(p^2*m)
             loss_c = 1 - 2*IK/(UP + UT + eps)
"""

import sys

import numpy as np

sys.path.insert(0, "/opt/trn_rl_repo")

import concourse.bass as bass  # noqa: E402
import concourse.tile as tile  # noqa: E402
from concourse import bacc, mybir  # noqa: E402
from concourse.bass_utils import run_bass_kernel_spmd  # noqa: E402

try:
    import ml_dtypes
    BF16_NP = ml_dtypes.bfloat16
except ImportError:  # pragma: no cover
    import jax.numpy as jnp
    BF16_NP = jnp.bfloat16

F32 = mybir.dt.float32
BF16 = mybir.dt.bfloat16
ALU = mybir.AluOpType
ACTF = mybir.ActivationFunctionType

B_PER_CORE = 2
N_CORES = 8
P = 128
FREE = 3200
SUBF = 800        # phase-1 subsample columns (1/4 of FREE, half partitions)
NITER = 8         # phase-1 bisection iterations
DELTA = 0.0085    # threshold safety margin in p-space (covers ~4.5 sigma of
                  # the 1/8-subsample quantile noise + bisect window)
EPS = 1e-6

# out_psum row map (row = 3*bank + base/32; halves A=0:256 B=256:512):
#  0: npos0|nneg0    1: npos1|nneg1    2: int0|int1
#  3: chi0|chi1      4: p2pos0|p2pos1  5: tsel0|tsel1
#  6..14: UTj|IKj (j=0..8)   15: UT9|IK9   16: UP0|UP1   17: UP2|UP3
#  18: UP4|UP5 (bank2 row 0, reused after its early flush)
PSUM_ROWS = 19
UPX_DVE = (0, 1, 2, 3, 4, 5)   # UP squares on DVE+PE; rest on ACT

# out_stats [128, 16] column map (host sums over partitions)
SC_UP = 0      # +j2 for ACT-UP planes (j2 not in UPX_DVE)
SC_MIDS = 10   # mids copy: rows 0 / 32 hold per-sample phase-1 estimate
SC_NCOL = 16


def build_bass(bench_iters=1, niter=NITER, wb=2, xb=4, tmb=3, pb=2, npool=0, pq_pool=False, ppp_pool=False, upx=UPX_DVE, p2pos_act=False):
    nc = bacc.Bacc("TRN2", target_bir_lowering=False, debug=False)

    pred = nc.dram_tensor("pred", [B_PER_CORE, 6, P, FREE], BF16,
                          kind="ExternalInput").ap()
    gtt = nc.dram_tensor("gt_text", [B_PER_CORE, P, FREE], BF16,
                         kind="ExternalInput").ap()
    gtk = nc.dram_tensor("gt_kernels", [B_PER_CORE, 5, P, FREE], BF16,
                         kind="ExternalInput").ap()
    msk = nc.dram_tensor("training_mask", [B_PER_CORE, P, FREE], BF16,
                         kind="ExternalInput").ap()
    out_psum = nc.dram_tensor("out_psum", [PSUM_ROWS, 512], F32,
                              kind="ExternalOutput").ap()
    out_stats = nc.dram_tensor("out_stats", [P, SC_NCOL], F32,
                               kind="ExternalOutput").ap()

    with tile.TileContext(nc) as tc:
        with (
            tc.tile_pool(name="pin", bufs=1) as pin,
            tc.tile_pool(name="stream", bufs=4) as stream,
            tc.tile_pool(name="work", bufs=wb) as work,
            tc.tile_pool(name="pacc", bufs=1, space="PSUM") as pacc,
            tc.tile_pool(name="pscr", bufs=1, space="PSUM") as pscr,
        ):
            if bench_iters > 1:
                loop_cm = tc.For_i(0, bench_iters, 1)
                loop_cm.__enter__()

            outs = pin.tile([P, SC_NCOL], F32, tag="outs")
            nc.vector.memset(outs, 0.0)

            # constant matmul weights. sample-b bisect state lives on
            # partition 32*b (engine partition bases must be 0/32/64).
            ones1 = pin.tile([P, 1], BF16, tag="ones1")
            nc.vector.memset(ones1, 1.0)
            bm2 = pin.tile([P, 33], BF16, tag="bm2")
            nc.vector.memset(bm2, 0.0)
            nc.vector.memset(bm2[0:64, 0:1], 1.0)
            nc.vector.memset(bm2[64:128, 32:33], 1.0)
            lbc = pin.tile([P, P], F32, tag="lbc")  # striped broadcast
            nc.vector.memset(lbc, 0.0)
            nc.vector.memset(lbc[0:1, 0:64], 1.0)
            nc.vector.memset(lbc[32:33, 64:128], 1.0)
            ab = [pin.tile([P, P], F32, tag=f"ab{b}", name=f"ab{b}")
                  for b in range(B_PER_CORE)]
            for b in range(B_PER_CORE):
                nc.vector.memset(ab[b], 0.0)
                nc.vector.memset(ab[b][32 * b:32 * b + 1, :], 1.0)

            # persistent PSUM accumulation banks
            banks = [pacc.tile([P, 512], F32, tag=f"bank{i}",
                               name=f"bank{i}") for i in range(6)]

            def pe_row(src, row, half):
                """Accumulate sum over partitions of src [P, FREE] into
                out_psum row `row`, half `half` (cols half*256 +: 256),
                via 13 chunked matmuls (last chunk 128 wide). Row 18
                physically reuses bank2 row 0 after its early flush."""
                bank, base = divmod(row if row < 18 else 6, 3)
                dst = banks[bank]
                off = half * 256
                for k in range(13):
                    w = 256 if k < 12 else 128
                    nc.tensor.matmul(
                        dst[base * 32:base * 32 + 1, off:off + w],
                        ones1, src[:, k * 256:k * 256 + w],
                        start=(k == 0), stop=(k == 12))

            # resident tiles
            m_t = [pin.tile([P, FREE], BF16, tag=f"m{b}", name=f"m{b}")
                   for b in range(B_PER_CORE)]
            pn_t = [pin.tile([P, FREE], BF16, tag=f"pn{b}", name=f"pn{b}")
                    for b in range(B_PER_CORE)]
            xg_t = []

            # bisection state (junk on unused partitions is kept finite)
            v2s = pin.tile([P, SUBF], BF16, tag="v2s")
            los = pin.tile([P, 1], F32, tag="los")
            his = pin.tile([P, 1], F32, tag="his")
            mids = pin.tile([P, 1], F32, tag="mids")
            ks = pin.tile([P, 1], F32, tag="ks")
            ksrc = pin.tile([P, 2], F32, tag="ksrc")
            cnt2 = pin.tile([P, 1], F32, tag="cnt2")
            cmp2 = pin.tile([P, 1], mybir.dt.uint32, tag="cmp2")
            his2 = pin.tile([P, 1], F32, tag="his2")
            va = pin.tile([P, 1], F32, tag="va")
            vb = pin.tile([P, 1], F32, tag="vb")
            scrA = pin.tile([P, 256], F32, tag="scrA")
            scr2 = pin.tile([33, 400], F32, tag="scr2")
            nc.vector.memset(los, 0.0)
            nc.vector.memset(his, 1.0)
            nc.vector.memset(mids, 0.5)
            nc.vector.memset(ks, 0.0)
            nc.vector.memset(ksrc, 0.0)
            nc.vector.memset(cnt2, 0.0)
            nc.vector.memset(his2, 0.0)

            planes = [(b, c) for b in range(B_PER_CORE) for c in range(5)]

            # text inputs first (the text phase is the critical-path head)
            for b in range(B_PER_CORE):
                nc.sync.dma_start(out=m_t[b], in_=msk[b])
                x = stream.tile([P, FREE], BF16, tag="x", name="xt", bufs=xb)
                nc.sync.dma_start(out=x, in_=pred[b, 0])
                g = stream.tile([P, FREE], BF16, tag="g", name="gt", bufs=2)
                nc.sync.dma_start(out=g, in_=gtt[b])
                xg_t.append((x, g))

            # Pool t*m pre-pass, emitted with ~3-plane lookahead: bootstrap
            # 3 here, the rest from inside the plane loop
            pool_js = set(range(10 - npool, 10))
            tm_t = {}

            def _pool_tm(j):
                if j not in pool_js:
                    return
                b, c = planes[j]
                t = stream.tile([P, FREE], BF16, tag="t", name="tk", bufs=3)
                nc.sync.dma_start(out=t, in_=gtk[b, c])
                tm = work.tile([P, FREE], BF16, tag="tmp", name="tmp",
                               bufs=2)
                nc.gpsimd.tensor_tensor(out=tm, in0=t, in1=m_t[b],
                                        op=ALU.mult)
                tm_t[j] = tm

            for j in range(10):
                if j < 10 - npool:
                    continue
                if len(tm_t) >= 3:
                    break
                _pool_tm(j)

            # ---------------- text phase ----------------
            for b in range(B_PER_CORE):
                x, g = xg_t[b]
                p = work.tile([P, FREE], BF16, tag="p", name="p", bufs=pb)
                nc.scalar.activation(out=p, in_=x, func=ACTF.Sigmoid)
                posm = work.tile([P, FREE], BF16, tag="posm", name="posm", bufs=pb)
                nc.vector.tensor_tensor(out=posm, in0=g, in1=m_t[b],
                                        op=ALU.mult)
                pe_row(posm, b, 0)             # npos_b
                negm = work.tile([P, FREE], BF16, tag="negm", name="negm")
                nc.vector.tensor_tensor(out=negm, in0=m_t[b], in1=posm,
                                        op=ALU.subtract)
                pe_row(negm, b, 1)             # nneg_b
                nc.vector.tensor_tensor(out=pn_t[b], in0=p, in1=negm,
                                        op=ALU.mult)
                pp = work.tile([P, FREE], BF16, tag="pp", name="pp", bufs=pb)
                nc.vector.tensor_tensor(out=pp, in0=p, in1=posm,
                                        op=ALU.mult)
                pe_row(pp, 2, b)               # intert_b
                ppp = work.tile([P, FREE], BF16, tag="negm", name="ppp",
                                bufs=2)
                nc.vector.tensor_tensor(out=ppp, in0=pp, in1=pp,
                                        op=ALU.mult)
                pe_row(ppp, 4, b)              # p2pos_b

            # ---- bisection chunks (interleaved with kernel planes) ----
            bis_chunks = []

            def _ksetup():
                # npos/nneg totals from bank0 half-rows -> va/vb
                nc.scalar.activation(out=scrA, in_=banks[0][:, 0:256],
                                     func=ACTF.Copy, accum_out=va)
                nc.scalar.activation(out=scrA, in_=banks[0][:, 256:512],
                                     func=ACTF.Copy, accum_out=vb)
                # (npos_b, nneg_b) onto partition 32b
                nc.sync.dma_start(out=ksrc[0:1, 0:1], in_=va[0:1, :])
                nc.sync.dma_start(out=ksrc[0:1, 1:2], in_=vb[0:1, :])
                nc.sync.dma_start(out=ksrc[32:33, 0:1], in_=va[32:33, :])
                nc.sync.dma_start(out=ksrc[32:33, 1:2], in_=vb[32:33, :])
            bis_chunks.append(_ksetup)

            def _ks():
                # ks = min(3*npos, nneg) / 8
                nc.vector.tensor_scalar(
                    out=ks, in0=ksrc[:, 0:1], scalar1=3.0,
                    scalar2=None, op0=ALU.mult)
                nc.vector.tensor_tensor(out=ks, in0=ks,
                                        in1=ksrc[:, 1:2], op=ALU.min)
                nc.vector.tensor_scalar(
                    out=ks, in0=ks, scalar1=0.125,
                    scalar2=None, op0=ALU.mult)
                # subsample: half partitions x first 800 cols
                nc.vector.tensor_copy(v2s[0:64, :], pn_t[0][0:64, 0:SUBF])
                nc.vector.tensor_copy(v2s[64:128, :],
                                      pn_t[1][64:128, 0:SUBF])
            bis_chunks.append(_ks)

            def _p1_iter():
                midb = pscr.tile([P, 1], F32, tag="midb", name="midb")
                nc.tensor.matmul(midb, lbc, mids, start=True, stop=True)
                sels = work.tile([P, SUBF], BF16, tag="sels", name="sels")
                nc.vector.tensor_scalar(
                    out=sels, in0=v2s, scalar1=midb, scalar2=None,
                    op0=ALU.is_ge)
                cntp = pscr.tile([33, 400], F32, tag="cntp", name="cntp")
                nc.tensor.matmul(cntp, bm2, sels[:, 0:400], start=True,
                                 stop=False)
                nc.tensor.matmul(cntp, bm2, sels[:, 400:800], start=False,
                                 stop=True)
                nc.scalar.activation(out=scr2, in_=cntp, func=ACTF.Copy,
                                     accum_out=cnt2[0:33, :])
                nc.vector.tensor_tensor(out=cmp2, in0=cnt2,
                                        in1=ks, op=ALU.is_ge)
                nc.vector.copy_predicated(out=los, mask=cmp2, data=mids)
                nc.vector.tensor_tensor(out=cmp2, in0=cnt2,
                                        in1=ks, op=ALU.is_lt)
                nc.vector.copy_predicated(out=his, mask=cmp2, data=mids)
                nc.vector.tensor_tensor(out=mids, in0=los,
                                        in1=his, op=ALU.add)
                nc.vector.tensor_scalar_mul(mids, mids, 0.5)
            bis_chunks.extend([_p1_iter] * niter)

            def _hi():
                nc.vector.tensor_scalar(
                    out=his2, in0=mids, scalar1=DELTA,
                    scalar2=None, op0=ALU.add)
                nc.vector.tensor_copy(outs[:, SC_MIDS:SC_MIDS + 1], mids)
            bis_chunks.append(_hi)

            def _final(b):
                hib = pscr.tile([P, 1], F32, tag="midb", name=f"hib{b}")
                nc.tensor.matmul(hib, ab[b], his2, start=True, stop=True)
                sel = work.tile([P, FREE], BF16, tag="sel", name="sel")
                nc.vector.tensor_scalar(
                    out=sel, in0=pn_t[b], scalar1=hib, scalar2=None,
                    op0=ALU.is_ge)
                pe_row(sel, 3, b)              # chi_b
                # tsel = sum(sel * pn^2): square on ACT, one product on DVE
                pn2 = work.tile([P, FREE], BF16, tag="w", name="pn2", bufs=1)
                nc.scalar.activation(out=pn2, in_=pn_t[b], func=ACTF.Square)
                w2 = work.tile([P, FREE], BF16, tag="sel", name="w2")
                nc.vector.tensor_tensor(out=w2, in0=sel, in1=pn2,
                                        op=ALU.mult)
                pe_row(w2, 5, b)               # tsel_b
            bis_chunks.append(lambda: _final(0))
            bis_chunks.append(lambda: _final(1))

            # ---------------- kernel planes (bisection interleaved) -----
            done_banks = set()

            def _flush_bank(i):
                scro = work.tile([P, 512], F32, tag="scro", name="scro")
                nc.scalar.activation(out=scro, in_=banks[i], func=ACTF.Copy)
                for r in range(3):
                    nc.sync.dma_start(
                        out=out_psum[i * 3 + r:i * 3 + r + 1, :],
                        in_=scro[32 * r:32 * r + 1, :])
                done_banks.add(i)

            emitted = 0
            for j, (b, c) in enumerate(planes):
                if j + 3 < len(planes) and (j + 3) not in tm_t:
                    _pool_tm(j + 3)
                xk = stream.tile([P, FREE], BF16, tag="x", name="xk", bufs=xb)
                nc.sync.dma_start(out=xk, in_=pred[b, c + 1])
                j2 = b * 5 + c

                pk = work.tile([P, FREE], BF16, tag="p", name="pk", bufs=pb)
                nc.scalar.activation(out=pk, in_=xk, func=ACTF.Sigmoid)
                if j2 not in tm_t:
                    t = stream.tile([P, FREE], BF16, tag="t", name="tk",
                                    bufs=3)
                    nc.sync.dma_start(out=t, in_=gtk[b, c])
                    tmj = work.tile([P, FREE], BF16, tag="tm", name="tm",
                                    bufs=tmb)
                    nc.vector.tensor_tensor(out=tmj, in0=t, in1=m_t[b],
                                            op=ALU.mult)
                    tm_t[j2] = tmj
                ikv = work.tile([P, FREE], BF16, tag="posm", name="ikv", bufs=pb)
                nc.vector.tensor_tensor(out=ikv, in0=tm_t[j2], in1=pk,
                                        op=ALU.mult)
                ut_row = 6 + j2
                pe_row(tm_t[j2], ut_row, 0)    # UT_j2
                pe_row(ikv, ut_row, 1)         # IK_j2
                pmtag = "pmkp" if (j2 in upx and pq_pool) else "pp"
                pmk = work.tile([P, FREE], BF16, tag=pmtag, name="pmk",
                                bufs=2 if pmtag == "pmkp" else pb)
                nc.vector.tensor_tensor(out=pmk, in0=pk, in1=m_t[b],
                                        op=ALU.mult)
                if j2 in upx:
                    pq = work.tile([P, FREE], BF16, tag="pq", name="pq",
                                   bufs=2)
                    if pq_pool:
                        nc.gpsimd.tensor_tensor(out=pq, in0=pmk, in1=pmk,
                                                op=ALU.mult)
                    else:
                        nc.vector.tensor_tensor(out=pq, in0=pmk, in1=pmk,
                                                op=ALU.mult)
                    ui = upx.index(j2)
                    pe_row(pq, (16, 17, 18)[ui // 2], ui % 2)
                else:
                    nc.scalar.activation(
                        out=pmk, in_=pmk, func=ACTF.Square,
                        accum_out=outs[:, SC_UP + j2:SC_UP + j2 + 1])
                # flush banks as their last accumulation completes.
                # bank2 (rows 6-8) flushes early; its row 0 is then reused
                # for UP4|UP5 (out_psum row 18, flushed at the end).
                if j2 == 0:
                    _flush_bank(0)   # npos/nneg/int rows (text)
                elif j2 == 3:
                    _flush_bank(2)   # rows 6-8 (j2 0-2)
                elif j2 == 6:
                    _flush_bank(3)   # rows 9-11 (j2 3-5)

                target = (j + 1) * len(bis_chunks) // len(planes)
                while emitted < target:
                    bis_chunks[emitted]()
                    emitted += 1
            while emitted < len(bis_chunks):
                bis_chunks[emitted]()
                emitted += 1

            # ---------------- output ----------------
            for i in (4, 5, 1, 3, 2, 0):
                if i not in done_banks:
                    _flush_bank(i)
            # second flush of bank2 row 0 (UP4|UP5)
            scr18 = work.tile([P, 512], F32, tag="scro", name="scr18")
            nc.scalar.activation(out=scr18[0:64, :], in_=banks[2][0:64, :],
                                 func=ACTF.Copy)
            nc.sync.dma_start(out=out_psum[18:19, :], in_=scr18[0:1, :])
            nc.sync.dma_start(out=out_stats, in_=outs)

            if bench_iters > 1:
                loop_cm.__exit__(None, None, None)

    nc.compile()
    return nc


_NC_CACHE = None


def _get_nc():
    global _NC_CACHE
    if _NC_CACHE is None:
        _NC_CACHE = build_bass()
    return _NC_CACHE


def _to_bf16(x):
    # vectorized round-to-nearest-even f32 -> bf16 (ml_dtypes astype is slow)
    u = np.ascontiguousarray(np.asarray(x, dtype=np.float32)).view(np.uint32)
    r = u + np.uint32(0x7FFF) + ((u >> np.uint32(16)) & np.uint32(1))
    return (r >> np.uint32(16)).astype(np.uint16).view(BF16_NP)


def make_in_maps(pred, gt_text, gt_kernels, training_mask):
    pred = _to_bf16(pred)
    gt_text = _to_bf16(gt_text)
    gt_kernels = _to_bf16(gt_kernels)
    training_mask = _to_bf16(training_mask)
    in_maps = []
    for core in range(N_CORES):
        s = slice(core * B_PER_CORE, (core + 1) * B_PER_CORE)
        in_maps.append({
            "pred": np.ascontiguousarray(pred[s]).reshape(
                B_PER_CORE, 6, P, FREE),
            "gt_text": np.ascontiguousarray(gt_text[s]).reshape(
                B_PER_CORE, P, FREE),
            "gt_kernels": np.ascontiguousarray(gt_kernels[s]).reshape(
                B_PER_CORE, 5, P, FREE),
            "training_mask": np.ascontiguousarray(training_mask[s]).reshape(
                B_PER_CORE, P, FREE),
        })
    return in_maps


def combine(core_outs):
    """core_outs: list of 8 (out_psum [18,512], out_stats [128,16])
    -> (loss, loss_text, loss_kernels)."""
    text_losses = []
    kernel_losses = []
    A, B = slice(0, 256), slice(256, 512)
    for op, os_ in core_outs:
        op = np.asarray(op, dtype=np.float64)
        os_ = np.asarray(os_, dtype=np.float64)
        npos = [op[0, A].sum(), op[1, A].sum()]
        nneg = [op[0, B].sum(), op[1, B].sum()]
        intert = [op[2, A].sum(), op[2, B].sum()]
        chi = [op[3, A].sum(), op[3, B].sum()]
        p2pos = [op[4, A].sum(), op[4, B].sum()]
        tsel = [op[5, A].sum(), op[5, B].sum()]
        ut = {}
        ik = {}
        for j2 in range(10):
            r = 6 + j2
            ut[j2] = op[r, A].sum()
            ik[j2] = op[r, B].sum()
        up = {}
        for j2 in UPX_DVE:
            ui = UPX_DVE.index(j2)
            up[j2] = op[(16, 17, 18)[ui // 2], A if ui % 2 == 0 else B].sum()
        for j2 in range(10):
            if j2 not in up:
                up[j2] = os_[:, SC_UP + j2].sum()
        mids = [os_[0, SC_MIDS], os_[32, SC_MIDS]]

        for b in range(B_PER_CORE):
            k = min(3.0 * npos[b], nneg[b])
            s = mids[b] + DELTA / 2.0
            T = tsel[b] + (k - chi[b]) * s * s
            union = p2pos[b] + T + npos[b] + EPS
            text_losses.append(1.0 - 2.0 * intert[b] / union)
            for c in range(5):
                j2 = b * 5 + c
                union_k = up[j2] + ut[j2] + EPS
                kernel_losses.append(1.0 - 2.0 * ik[j2] / union_k)
    loss_text = float(np.mean(text_losses))
    loss_kernels = float(np.mean(kernel_losses))
    loss = loss_kernels + 0.5 * loss_text
    return (np.float32(loss), np.float32(loss_text), np.float32(loss_kernels))


def kernel(pred, gt_text, gt_kernels, training_mask):
    nc = _get_nc()
    in_maps = make_in_maps(pred, gt_text, gt_kernels, training_mask)
    res = run_bass_kernel_spmd(nc, in_maps, core_ids=list(range(N_CORES)))
    core_outs = [(res.results[i]["out_psum"], res.results[i]["out_stats"])
                 for i in range(N_CORES)]
    return combine(core_outs)


if __name__ == "__main__":
    rng = np.random.default_rng(0)
    B, C, H, W = 16, 6, 640, 640
    pred = rng.standard_normal((B, C, H, W), dtype=np.float32)
    gt_text = (rng.random((B, 1, H, W)) > 0.9).astype(np.float32)
    gt_kernels = (rng.random((B, C - 1, H, W)) > 0.9).astype(np.float32)
    training_mask = (rng.random((B, 1, H, W)) > 0.05).astype(np.float32)
    print(kernel(pred, gt_text, gt_kernels, training_mask))
